# revision 5
# baseline (speedup 1.0000x reference)
"""Trainium2 Bass kernel for ContextualAttention (sparse_attention).

Reference computation (per sample b of X:(4,128,64,64)):
  Xd = X[:, :, ::2, ::2]                                  (128,32,32)
  key/query patches = 3x3 SAME patches of Xd              L=P=1024, D=128*9
  scores[l,p] = <patch_p, patch_l / max(||patch_l||,eps)>
  logits = scores * mm[l] * 10 ; yi = softmax_l(logits) * mm[l]
  deconv: T[c,p,u,v] = sum_l yi[l,p] * DW[l,c,u,v],  DW from 4x4/stride2
          SAME patches of X; overlap-add into 66x66 canvas, crop/4.

Sharding: pure data parallel; core = (sample b = core//2, half ph = core%2)
owning 512 query positions (16 of 32 patch rows). Keys/l are full 1024 on
every core (softmax over l stays local). Canvas halves overlap by 2 rows,
summed on host during unshard.

Device program (identical SPMD on all 8 cores, per-core data):
  scores: 9 shifted-window fp32 matmuls accumulated in PSUM
          (contract dim = channels; one (u,v) tap at a time)
  softmax: PSUM eviction fused with per-key scale/mask + running max
          (tensor_tensor_reduce), exp with accumulated sum (ACT), then
          (e * 1/Z) * mm in one scalar_tensor_tensor
  yi transpose: PE transposes (l on partitions for the deconv contract)
  deconv: 16 (u,v) taps x 8 l-chunks fp32 matmuls; weights DMA-gathered
          from padded X^T/4 with a strided access pattern
  overlap-add: 16 strided vector adds into a (128, 34x66) canvas tile
"""

import sys

if "/opt/trn_rl_repo" not in sys.path:
    sys.path.insert(0, "/opt/trn_rl_repo")

import numpy as np

B = 4
C = 128
H = 64
HD = 32          # downsampled h=w
L = HD * HD      # 1024 key patches / positions
SCALE = 10.0
EPS = 1e-4
N_CORES = 8


def _build_program():
    import concourse.mybir as mybir
    import concourse.tile as tile
    from concourse import bacc
    from concourse.masks import make_identity

    f32 = mybir.dt.float32
    Alu = mybir.AluOpType
    Act = mybir.ActivationFunctionType

    nc = bacc.Bacc("TRN2", debug=False, num_devices=N_CORES)

    xdpad_d = nc.dram_tensor("xdpad", (C, 34, 34), f32, kind="ExternalInput")
    xqpad_d = nc.dram_tensor("xqpad", (C, 18, 34), f32, kind="ExternalInput")
    xt4_d = nc.dram_tensor("xt4", (33, 2, 33, 2, C), f32, kind="ExternalInput")
    sscale_d = nc.dram_tensor("sscale", (1, L), f32, kind="ExternalInput")
    mmrow_d = nc.dram_tensor("mmrow", (1, L), f32, kind="ExternalInput")
    canvas_d = nc.dram_tensor("canvas", (C, 17, 2, 33, 2), f32, kind="ExternalOutput")

    uvs3 = [(u, v) for u in range(3) for v in range(3)]
    uvs4 = [(u, v) for u in range(4) for v in range(4)]

    with tile.TileContext(nc) as tc:
        with (
            tc.tile_pool(name="const", bufs=1) as cpool,
            tc.tile_pool(name="work", bufs=2) as wpool,
            tc.tile_pool(name="dw", bufs=3) as dwpool,
            tc.tile_pool(name="ps", bufs=2, space="PSUM") as pspool,
            tc.tile_pool(name="pt", bufs=2, space="PSUM") as ptpool,
            tc.tile_pool(name="pd", bufs=2, space="PSUM") as pdpool,
        ):
            ident = cpool.tile([128, 128], f32, tag="ident")
            make_identity(nc, ident)

            xdp = cpool.tile([C, 34, 34], f32, tag="xdp")
            nc.sync.dma_start(out=xdp, in_=xdpad_d.ap())
            xqp = cpool.tile([C, 18, 34], f32, tag="xqp")
            nc.sync.dma_start(out=xqp, in_=xqpad_d.ap())
            ssb = cpool.tile([128, L], f32, tag="ssb")
            nc.gpsimd.dma_start(out=ssb, in_=sscale_d.ap().to_broadcast((128, L)))
            mmb = cpool.tile([128, L], f32, tag="mmb")
            nc.gpsimd.dma_start(out=mmb, in_=mmrow_d.ap().to_broadcast((128, L)))

            # Dense shifted windows of the padded downsampled features:
            # kuv[(u,v)][c, i, j] = Xd_pad[c, i+u, j+v]  (keys, all 1024)
            # quv[(u,v)][c, i, j] = Xq_pad[c, i+u, j+v]  (this core's 512)
            kuv = {}
            quv = {}
            for u, v in uvs3:
                kt = cpool.tile([C, 32, 32], f32, tag=f"k{u}{v}")
                nc.vector.tensor_copy(out=kt, in_=xdp[:, u : u + 32, v : v + 32])
                kuv[(u, v)] = kt
                qt = cpool.tile([C, 16, 32], f32, tag=f"q{u}{v}")
                nc.scalar.copy(out=qt, in_=xqp[:, u : u + 16, v : v + 32])
                quv[(u, v)] = qt

            # yiT[l % 128, l // 128, p] = yi[p, l]
            yiT = cpool.tile([128, 8, 512], f32, tag="yiT")

            for pc in range(4):
                ps0 = pspool.tile([128, 512], f32, tag="ps0")
                ps1 = pspool.tile([128, 512], f32, tag="ps1")
                for idx, (u, v) in enumerate(uvs3):
                    lhsT = quv[(u, v)][:, pc * 4 : (pc + 1) * 4, :]
                    st = idx == 0
                    sp = idx == len(uvs3) - 1
                    nc.tensor.matmul(ps0, lhsT, kuv[(u, v)][:, 0:16, :], start=st, stop=sp)
                    nc.tensor.matmul(ps1, lhsT, kuv[(u, v)][:, 16:32, :], start=st, stop=sp)

                # logits = raw * (SCALE * mm / max(norm, EPS)); then row max
                # (tensor_tensor_reduce is broken on the axon HW path, so
                # evict with tensor_mul and reduce separately)
                s_sb = wpool.tile([128, L], f32, tag="s")
                mx = wpool.tile([128, 1], f32, tag="mx")
                nc.vector.tensor_mul(out=s_sb[:, 0:512], in0=ps0, in1=ssb[:, 0:512])
                nc.vector.tensor_mul(
                    out=s_sb[:, 512:1024], in0=ps1, in1=ssb[:, 512:1024]
                )
                nc.vector.reduce_max(out=mx, in_=s_sb, axis=mybir.AxisListType.X)
                negmx = wpool.tile([128, 1], f32, tag="negmx")
                nc.scalar.mul(negmx, mx, -1.0)
                e_sb = wpool.tile([128, L], f32, tag="e")
                zs = wpool.tile([128, 1], f32, tag="zs")
                nc.scalar.activation(
                    out=e_sb, in_=s_sb, func=Act.Exp,
                    bias=negmx, scale=1.0, accum_out=zs,
                )
                rz = wpool.tile([128, 1], f32, tag="rz")
                nc.vector.reciprocal(rz, zs)
                yi = wpool.tile([128, L], f32, tag="yi")
                nc.vector.scalar_tensor_tensor(
                    out=yi, in0=e_sb, scalar=rz, in1=mmb,
                    op0=Alu.mult, op1=Alu.mult,
                )
                for lc in range(8):
                    pt = ptpool.tile([128, 128], f32, tag="pt")
                    nc.tensor.transpose(pt, yi[:, lc * 128 : (lc + 1) * 128], ident)
                    nc.scalar.copy(out=yiT[:, lc, pc * 128 : (pc + 1) * 128], in_=pt)

            canvas = cpool.tile([C, 17, 2, 33, 2], f32, tag="canvas")
            nc.vector.memset(canvas, 0.0)
            for u, v in uvs4:
                dw = dwpool.tile([128, 8, 128], f32, tag="dw")
                src = xt4_d.ap()[
                    u // 2 : u // 2 + 32, u % 2, v // 2 : v // 2 + 32, v % 2, :
                ]  # (i=32, j=32, c=128)
                for lc in range(8):
                    nc.sync.dma_start(
                        out=dw[:, lc, :], in_=src[lc * 4 : (lc + 1) * 4]
                    )
                pd = pdpool.tile([128, 512], f32, tag="pd")
                for lc in range(8):
                    nc.tensor.matmul(
                        pd, dw[:, lc, :], yiT[:, lc, :],
                        start=(lc == 0), stop=(lc == 7),
                    )
                dst = canvas[:, u // 2 : u // 2 + 16, u % 2, v // 2 : v // 2 + 32, v % 2]
                nc.vector.tensor_add(
                    out=dst, in0=dst, in1=pd.rearrange("p (a b) -> p a b", a=16)
                )
            nc.sync.dma_start(out=canvas_d.ap(), in_=canvas)

    nc.compile()
    return nc


def prep_core_inputs(X, mask):
    """Host-side sharding: build the per-core input maps."""
    X = np.asarray(X, dtype=np.float32)
    mask = np.asarray(mask, dtype=np.float32)

    Xd = X[:, :, ::2, ::2]  # (B,128,32,32)
    Xdp = np.zeros((B, C, 34, 34), np.float32)
    Xdp[:, :, 1:33, 1:33] = Xd

    # key-patch validity from mask sample 0 only (as in reference)
    md0 = mask[0, 0, ::8, ::8]  # (32,32)
    md0p = np.zeros((34, 34), np.float32)
    md0p[1:33, 1:33] = md0
    win = np.zeros((32, 32), np.float32)
    for du in range(3):
        for dv in range(3):
            win += md0p[du : du + 32, dv : dv + 32]
    mm = (win == 0).astype(np.float32).reshape(-1)  # (1024,)

    # per-sample l2 norms of key patches (zero padding included)
    S = (Xdp * Xdp).sum(axis=1)  # (B,34,34)
    n2 = np.zeros((B, 32, 32), np.float32)
    for du in range(3):
        for dv in range(3):
            n2 += S[:, du : du + 32, dv : dv + 32]
    norms = np.sqrt(n2).reshape(B, L)
    sscale = (SCALE * mm[None, :] / np.maximum(norms, EPS)).astype(np.float32)

    # padded X^T / 4 for the deconv weight gather
    Xp66 = np.zeros((B, C, 66, 66), np.float32)
    Xp66[:, :, 1:65, 1:65] = X * 0.25
    xt4 = np.ascontiguousarray(
        Xp66.reshape(B, C, 33, 2, 33, 2).transpose(0, 2, 3, 4, 5, 1)
    )  # (B,33,2,33,2,128)

    in_maps = []
    for core in range(N_CORES):
        b, ph = core // 2, core % 2
        in_maps.append(
            {
                "xdpad": np.ascontiguousarray(Xdp[b]),
                "xqpad": np.ascontiguousarray(Xdp[b][:, 16 * ph : 16 * ph + 18, :]),
                "xt4": xt4[b],
                "sscale": sscale[b : b + 1],
                "mmrow": np.ascontiguousarray(mm[None, :]),
            }
        )
    return in_maps


def assemble_output(core_canvases):
    """Gather/unshard: overlap-add the per-core half canvases, crop."""
    Canvas = np.zeros((B, C, 66, 66), np.float32)
    for core in range(N_CORES):
        b, ph = core // 2, core % 2
        Canvas[b, :, 32 * ph : 32 * ph + 34, :] += core_canvases[core].reshape(
            C, 34, 66
        )
    return np.ascontiguousarray(Canvas[:, :, 1:65, 1:65])


def kernel(X, mask):
    from concourse.bass_utils import run_bass_kernel_spmd

    nc = _build_program()
    in_maps = prep_core_inputs(X, mask)
    res = run_bass_kernel_spmd(nc, in_maps, core_ids=list(range(N_CORES)))
    return assemble_output([res.results[c]["canvas"] for c in range(N_CORES)])


# revision 8
# speedup vs baseline: 640.0966x; 640.0966x over previous
"""Trainium2 Bass kernel for ContextualAttention (sparse_attention).

Reference computation (per sample b of X:(4,128,64,64)):
  Xd = X[:, :, ::2, ::2]                                  (128,32,32)
  key/query patches = 3x3 SAME patches of Xd              L=P=1024, D=128*9
  scores[l,p] = <patch_p, patch_l / max(||patch_l||,eps)>
  logits = scores * mm[l] * 10 ; yi = softmax_l(logits) * mm[l]
  deconv: T[c,p,u,v] = sum_l yi[l,p] * DW[l,c,u,v],  DW from 4x4/stride2
          SAME patches of X; overlap-add into 66x66 canvas, crop/4.

Sharding: pure data parallel; core = (sample b = core//2, half ph = core%2)
owning 512 query positions (16 of 32 patch rows). Keys/l are full 1024 on
every core (softmax over l stays local). Canvas halves overlap by 2 rows,
summed on host during unshard.

Device program (identical SPMD on all 8 cores, per-core data):
  scores: 9 shifted-window fp32 matmuls accumulated in PSUM
          (contract dim = channels; one (u,v) tap at a time)
  softmax: PSUM eviction fused with per-key scale/mask + running max
          (tensor_tensor_reduce), exp with accumulated sum (ACT), then
          (e * 1/Z) * mm in one scalar_tensor_tensor
  yi transpose: PE transposes (l on partitions for the deconv contract)
  deconv: 16 (u,v) taps x 8 l-chunks fp32 matmuls; weights DMA-gathered
          from padded X^T/4 with a strided access pattern
  overlap-add: 16 strided vector adds into a (128, 34x66) canvas tile
"""

import sys

if "/opt/trn_rl_repo" not in sys.path:
    sys.path.insert(0, "/opt/trn_rl_repo")

import numpy as np

B = 4
C = 128
H = 64
HD = 32          # downsampled h=w
L = HD * HD      # 1024 key patches / positions
SCALE = 10.0
EPS = 1e-4
N_CORES = 8


def _build_program(loops=1):
    """Build the SPMD per-core program. loops>1 wraps the whole compute in a
    device-side For_i so steady-state per-iteration time can be measured
    (timing only; the production kernel uses loops=1 with no loop overhead)."""
    import concourse.mybir as mybir
    import concourse.tile as tile
    from concourse import bacc
    from concourse.masks import make_identity

    f32 = mybir.dt.float32
    Alu = mybir.AluOpType
    Act = mybir.ActivationFunctionType

    nc = bacc.Bacc("TRN2", debug=False, num_devices=N_CORES)

    xdpad_d = nc.dram_tensor("xdpad", (C, 34, 34), f32, kind="ExternalInput")
    xqpad_d = nc.dram_tensor("xqpad", (C, 18, 34), f32, kind="ExternalInput")
    xt4_d = nc.dram_tensor("xt4", (33, 2, 33, 2, C), f32, kind="ExternalInput")
    sscale_d = nc.dram_tensor("sscale", (1, L), f32, kind="ExternalInput")
    mmrow_d = nc.dram_tensor("mmrow", (1, L), f32, kind="ExternalInput")
    canvas_d = nc.dram_tensor("canvas", (C, 17, 2, 33, 2), f32, kind="ExternalOutput")

    uvs3 = [(u, v) for u in range(3) for v in range(3)]
    uvs4 = [(u, v) for u in range(4) for v in range(4)]

    with tile.TileContext(nc) as tc:
        with (
            tc.tile_pool(name="const", bufs=1) as cpool,
            tc.tile_pool(name="work", bufs=2) as wpool,
            tc.tile_pool(name="dw", bufs=3) as dwpool,
            tc.tile_pool(name="ps", bufs=2, space="PSUM") as pspool,
            tc.tile_pool(name="pt", bufs=2, space="PSUM") as ptpool,
            tc.tile_pool(name="pd", bufs=2, space="PSUM") as pdpool,
        ):
            ident = cpool.tile([128, 128], f32, tag="ident")
            make_identity(nc, ident)

            def emit_body():
                _emit_body(
                    nc, tc, mybir, f32, Alu, Act,
                    cpool, wpool, dwpool, pspool, ptpool, pdpool,
                    ident, xdpad_d, xqpad_d, xt4_d, sscale_d, mmrow_d, canvas_d,
                    uvs3, uvs4,
                )

            if loops == 1:
                emit_body()
            else:
                with tc.For_i(0, loops, 1):
                    emit_body()

    nc.compile()
    return nc


def _emit_body(nc, tc, mybir, f32, Alu, Act, cpool, wpool, dwpool, pspool,
               ptpool, pdpool, ident, xdpad_d, xqpad_d, xt4_d, sscale_d,
               mmrow_d, canvas_d, uvs3, uvs4):
            xdp = cpool.tile([C, 34, 34], f32, tag="xdp")
            nc.sync.dma_start(out=xdp, in_=xdpad_d.ap())
            xqp = cpool.tile([C, 18, 34], f32, tag="xqp")
            nc.sync.dma_start(out=xqp, in_=xqpad_d.ap())
            ssb = cpool.tile([128, L], f32, tag="ssb")
            nc.gpsimd.dma_start(out=ssb, in_=sscale_d.ap().to_broadcast((128, L)))
            mmb = cpool.tile([128, L], f32, tag="mmb")
            nc.gpsimd.dma_start(out=mmb, in_=mmrow_d.ap().to_broadcast((128, L)))

            # Dense shifted windows of the padded downsampled features:
            # kuv[(u,v)][c, i, j] = Xd_pad[c, i+u, j+v]  (keys, all 1024)
            # quv[(u,v)][c, i, j] = Xq_pad[c, i+u, j+v]  (this core's 512)
            kuv = {}
            quv = {}
            for u, v in uvs3:
                kt = cpool.tile([C, 32, 32], f32, tag=f"k{u}{v}")
                nc.vector.tensor_copy(out=kt, in_=xdp[:, u : u + 32, v : v + 32])
                kuv[(u, v)] = kt
                qt = cpool.tile([C, 16, 32], f32, tag=f"q{u}{v}")
                nc.scalar.copy(out=qt, in_=xqp[:, u : u + 16, v : v + 32])
                quv[(u, v)] = qt

            # yiT[l % 128, l // 128, p] = yi[p, l]
            yiT = cpool.tile([128, 8, 512], f32, tag="yiT")

            for pc in range(4):
                ps0 = pspool.tile([128, 512], f32, tag="ps0")
                ps1 = pspool.tile([128, 512], f32, tag="ps1")
                for idx, (u, v) in enumerate(uvs3):
                    lhsT = quv[(u, v)][:, pc * 4 : (pc + 1) * 4, :]
                    st = idx == 0
                    sp = idx == len(uvs3) - 1
                    nc.tensor.matmul(ps0, lhsT, kuv[(u, v)][:, 0:16, :], start=st, stop=sp)
                    nc.tensor.matmul(ps1, lhsT, kuv[(u, v)][:, 16:32, :], start=st, stop=sp)

                # logits = raw * (SCALE * mm / max(norm, EPS)); then row max
                # (tensor_tensor_reduce is broken on the axon HW path, so
                # evict with tensor_mul and reduce separately)
                s_sb = wpool.tile([128, L], f32, tag="s")
                mx = wpool.tile([128, 1], f32, tag="mx")
                nc.vector.tensor_mul(out=s_sb[:, 0:512], in0=ps0, in1=ssb[:, 0:512])
                nc.vector.tensor_mul(
                    out=s_sb[:, 512:1024], in0=ps1, in1=ssb[:, 512:1024]
                )
                nc.vector.reduce_max(out=mx, in_=s_sb, axis=mybir.AxisListType.X)
                negmx = wpool.tile([128, 1], f32, tag="negmx")
                nc.scalar.mul(negmx, mx, -1.0)
                e_sb = wpool.tile([128, L], f32, tag="e")
                zs = wpool.tile([128, 1], f32, tag="zs")
                nc.scalar.activation(
                    out=e_sb, in_=s_sb, func=Act.Exp,
                    bias=negmx, scale=1.0, accum_out=zs,
                )
                rz = wpool.tile([128, 1], f32, tag="rz")
                nc.vector.reciprocal(rz, zs)
                yi = wpool.tile([128, L], f32, tag="yi")
                nc.vector.scalar_tensor_tensor(
                    out=yi, in0=e_sb, scalar=rz, in1=mmb,
                    op0=Alu.mult, op1=Alu.mult,
                )
                for lc in range(8):
                    pt = ptpool.tile([128, 128], f32, tag="pt")
                    nc.tensor.transpose(pt, yi[:, lc * 128 : (lc + 1) * 128], ident)
                    nc.scalar.copy(out=yiT[:, lc, pc * 128 : (pc + 1) * 128], in_=pt)

            canvas = cpool.tile([C, 17, 2, 33, 2], f32, tag="canvas")
            nc.vector.memset(canvas, 0.0)
            for u, v in uvs4:
                dw = dwpool.tile([128, 8, 128], f32, tag="dw")
                src = xt4_d.ap()[
                    u // 2 : u // 2 + 32, u % 2, v // 2 : v // 2 + 32, v % 2, :
                ]  # (i=32, j=32, c=128)
                for lc in range(8):
                    nc.sync.dma_start(
                        out=dw[:, lc, :], in_=src[lc * 4 : (lc + 1) * 4]
                    )
                pd = pdpool.tile([128, 512], f32, tag="pd")
                for lc in range(8):
                    nc.tensor.matmul(
                        pd, dw[:, lc, :], yiT[:, lc, :],
                        start=(lc == 0), stop=(lc == 7),
                    )
                dst = canvas[:, u // 2 : u // 2 + 16, u % 2, v // 2 : v // 2 + 32, v % 2]
                nc.vector.tensor_add(
                    out=dst, in0=dst, in1=pd.rearrange("p (a b) -> p a b", a=16)
                )
            nc.sync.dma_start(out=canvas_d.ap(), in_=canvas)


def prep_core_inputs(X, mask):
    """Host-side sharding: build the per-core input maps."""
    X = np.asarray(X, dtype=np.float32)
    mask = np.asarray(mask, dtype=np.float32)

    Xd = X[:, :, ::2, ::2]  # (B,128,32,32)
    Xdp = np.zeros((B, C, 34, 34), np.float32)
    Xdp[:, :, 1:33, 1:33] = Xd

    # key-patch validity from mask sample 0 only (as in reference)
    md0 = mask[0, 0, ::8, ::8]  # (32,32)
    md0p = np.zeros((34, 34), np.float32)
    md0p[1:33, 1:33] = md0
    win = np.zeros((32, 32), np.float32)
    for du in range(3):
        for dv in range(3):
            win += md0p[du : du + 32, dv : dv + 32]
    mm = (win == 0).astype(np.float32).reshape(-1)  # (1024,)

    # per-sample l2 norms of key patches (zero padding included)
    S = (Xdp * Xdp).sum(axis=1)  # (B,34,34)
    n2 = np.zeros((B, 32, 32), np.float32)
    for du in range(3):
        for dv in range(3):
            n2 += S[:, du : du + 32, dv : dv + 32]
    norms = np.sqrt(n2).reshape(B, L)
    sscale = (SCALE * mm[None, :] / np.maximum(norms, EPS)).astype(np.float32)

    # padded X^T / 4 for the deconv weight gather
    Xp66 = np.zeros((B, C, 66, 66), np.float32)
    Xp66[:, :, 1:65, 1:65] = X * 0.25
    xt4 = np.ascontiguousarray(
        Xp66.reshape(B, C, 33, 2, 33, 2).transpose(0, 2, 3, 4, 5, 1)
    )  # (B,33,2,33,2,128)

    in_maps = []
    for core in range(N_CORES):
        b, ph = core // 2, core % 2
        in_maps.append(
            {
                "xdpad": np.ascontiguousarray(Xdp[b]),
                "xqpad": np.ascontiguousarray(Xdp[b][:, 16 * ph : 16 * ph + 18, :]),
                "xt4": xt4[b],
                "sscale": sscale[b : b + 1],
                "mmrow": np.ascontiguousarray(mm[None, :]),
            }
        )
    return in_maps


def assemble_output(core_canvases):
    """Gather/unshard: overlap-add the per-core half canvases, crop."""
    Canvas = np.zeros((B, C, 66, 66), np.float32)
    for core in range(N_CORES):
        b, ph = core // 2, core % 2
        Canvas[b, :, 32 * ph : 32 * ph + 34, :] += core_canvases[core].reshape(
            C, 34, 66
        )
    return np.ascontiguousarray(Canvas[:, :, 1:65, 1:65])


def kernel(X, mask):
    from concourse.bass_utils import run_bass_kernel_spmd

    nc = _build_program()
    in_maps = prep_core_inputs(X, mask)
    res = run_bass_kernel_spmd(nc, in_maps, core_ids=list(range(N_CORES)))
    return assemble_output([res.results[c]["canvas"] for c in range(N_CORES)])
